# revision 1
# baseline (speedup 1.0000x reference)
"""DIN-attention kernel for Trainium2, 8-core SPMD.

Reference computation (per batch b, seq pos l, x = item_seq[b, l]):
    mlp_in = [tgt, x, x-tgt, x*tgt]           (4D = 512)
    h      = relu(mlp_in @ W1 + b1)           (2D = 256)
    score  = h @ W2 + b2                      (1)
    out_b  = sum_l score[l] * x[l] * (l < seq_len[b])

Algebraic restructure (W1 = [A; B; C; Dm] in 128-row blocks):
    z   = x @ (B + C) + (x*tgt) @ Dm + c_b,   c_b = tgt_b @ (A - C) + b1
    out = sum_{l < n_b} (W2.T relu(z) + b2) * x[l]

Device strategy (per core):
  - Batches sorted by seq_len descending; slot s holds global ranks
    [8s, 8s+8), one per core, padded to a shared per-slot length L_s
    (max over the 8, rounded even).  Zero-padded columns contribute
    exactly 0 to the output, so padding is safe, and all 8 cores run an
    identical (SPMD) program while loading only ~half the dense bytes.
  - Tokens packed host-side into a transposed (128=D, T) fp32 array per
    core; compute in the "hidden-on-partitions" layout:
      zT (128=hid_half, n) = Wbc_h.T @ X + Wd_h.T @ Y + Cwin_h.T @ IND
    with Y = X * tgt_col (per-slot, gpsimd) and IND a host-built 0/1
    (32, T) slot-window indicator; Cwin packs the c_b bias rows.
  - relu on ScalarE, then score broadcast to all 128 partitions in one
    PSUM accumulation: P = W2rep0.T @ r0 + W2rep1.T @ r1 + b2 * ones
    (W2rep[k, m] = W2[k] for every m, so every output row = score row).
  - Final per-slot reduce: fused DVE tensor_tensor_reduce
    acc[:, s] = sum_cols (X * P), chained across 512-tiles via initial.
  - Matmuls run in float32r (fp32 storage, single-pass PE streaming).
"""

import sys

import numpy as np

for _p in ("/opt/trn_rl_repo",):
    if _p not in sys.path:
        sys.path.insert(0, _p)

import concourse.bacc as bacc
import concourse.bass as bass
import concourse.tile as tile
from concourse import mybir
from concourse.bass_utils import run_bass_kernel_spmd

assert bass  # re-exported for callers

B_FULL = 2048
L_FULL = 200
D = 128
N_CORES = 8
HID = 256  # 2D
TILE_N = 512  # fp32 PSUM bank columns
CHUNK_TARGET = 8192  # tokens per streamed chunk (slot-aligned)
F32 = mybir.dt.float32
F32R = mybir.dt.float32r
BF16 = mybir.dt.bfloat16

HOST_Y_BF16 = True  # ship Y = X*tgt as a host-packed bf16 array
RELU_BF16 = False  # bf16 ACT output is broken on TRN2 HW (probe E); use f32r
REDUCE_MODE = "stt"  # "ttr" | "stt" | "ttred"  (final per-slot reduce impl)
XB_BF16 = True  # ship X itself in bf16 (halves X DMA; bf16 h-matmuls)
FIRST_CHUNK = 4096  # smaller first chunk to hide pipeline fill
STREAM_BUFS = 2  # chunk-level double buffering
RB_BUFS = 2  # relu/dump tile buffering
B2VAL = [0.0]  # b2 constant, set by build_all before tracing


def _plan(seq_len):
    """Slot plan shared by all cores (SPMD: identical program)."""
    n = np.clip(np.asarray(seq_len).astype(np.int64), 0, L_FULL)
    order = np.argsort(-n, kind="stable")  # descending
    n_sorted = n[order]
    slot_lens = []
    for s in range(B_FULL // N_CORES):
        m = int(n_sorted[N_CORES * s])  # max of ranks [8s, 8s+8)
        if m <= 0:
            break
        slot_lens.append(m + (m & 1))  # round up to even
    S = len(slot_lens)
    offs = np.zeros(S + 1, dtype=np.int64)
    offs[1:] = np.cumsum(slot_lens)
    T = int(offs[-1])

    # chunks: contiguous slot ranges with <= CHUNK_TARGET tokens.
    # The first chunk is smaller so compute starts before the bulk DMA.
    chunks = []  # (slot_a, slot_b, tok_off, tok_len)
    sa = 0
    while sa < S:
        cap = FIRST_CHUNK if not chunks else CHUNK_TARGET
        sb = sa
        while sb < S and offs[sb + 1] - offs[sa] <= cap:
            sb += 1
        if sb == sa:
            sb = sa + 1
        chunks.append((sa, sb, int(offs[sa]), int(offs[sb] - offs[sa])))
        sa = sb
    return n, order, slot_lens, offs, T, chunks


def _build_program(slot_lens, offs, T, chunks):
    S = len(slot_lens)
    NW = (S + 31) // 32  # 32-slot bias windows
    nc = bacc.Bacc("TRN2", target_bir_lowering=False, debug=False)

    RDT = BF16 if RELU_BF16 else F32R
    YDT = BF16 if HOST_Y_BF16 else F32
    XDT = BF16 if XB_BF16 else F32R

    xt_d = nc.dram_tensor("xt", [D, T], XDT, kind="ExternalInput")
    ind_d = nc.dram_tensor("ind", [32, T], BF16, kind="ExternalInput")
    if HOST_Y_BF16:
        yb_d = nc.dram_tensor("yb", [D, T], BF16, kind="ExternalInput")
    else:
        tgt_d = nc.dram_tensor("tgt", [D, S], F32, kind="ExternalInput")
    cbw_d = nc.dram_tensor("cbw", [32, NW * HID], BF16, kind="ExternalInput")
    wbc_d = nc.dram_tensor("wbc", [D, HID], XDT, kind="ExternalInput")
    wd_d = nc.dram_tensor("wd", [D, HID], YDT, kind="ExternalInput")
    w2r_d = nc.dram_tensor("w2r", [D, HID], RDT, kind="ExternalInput")
    out_d = nc.dram_tensor("out_t", [D, 256], F32, kind="ExternalOutput")

    cmax = max(c[3] for c in chunks)

    with tile.TileContext(nc) as tc:
        with (
            tc.tile_pool(name="const", bufs=1) as cpool,
            tc.tile_pool(name="xst", bufs=STREAM_BUFS) as xpool,
            tc.tile_pool(name="yst", bufs=STREAM_BUFS) as ypool,
            tc.tile_pool(name="ist", bufs=STREAM_BUFS) as ipool,
            tc.tile_pool(name="rst", bufs=RB_BUFS) as rpool,
            tc.tile_pool(name="dst", bufs=RB_BUFS) as dpool,
            tc.tile_pool(name="ps", bufs=2, space="PSUM") as pspool,
        ):
            wbc = cpool.tile([D, HID], XDT, tag="wbc")
            wd = cpool.tile([D, HID], YDT, tag="wd")
            w2r = cpool.tile([D, HID], RDT, tag="w2r")
            cbw = cpool.tile([32, NW * HID], BF16, tag="cbw")
            acc = cpool.tile([D, 256], F32, tag="acc")
            aux = cpool.tile([D, 2], F32, tag="aux")

            nc.sync.dma_start(out=wbc[:], in_=wbc_d[:])
            nc.sync.dma_start(out=wd[:], in_=wd_d[:])
            nc.sync.dma_start(out=w2r[:], in_=w2r_d[:])
            if not HOST_Y_BF16:
                tgt = cpool.tile([D, S], F32, tag="tgt")
                nc.sync.dma_start(out=tgt[:], in_=tgt_d[:])
            nc.sync.dma_start(out=cbw[:], in_=cbw_d[:])
            nc.vector.memset(acc[:], 0.0)

            for sa, sb, toff, tlen in chunks:
                x = xpool.tile([D, cmax], XDT, tag="x")
                y = ypool.tile([D, cmax], YDT, tag="y")
                indt = ipool.tile([32, cmax], BF16, tag="ind")
                nc.sync.dma_start(out=x[:, :tlen], in_=xt_d[:, toff : toff + tlen])
                nc.sync.dma_start(out=indt[:, :tlen], in_=ind_d[:, toff : toff + tlen])

                if HOST_Y_BF16:
                    nc.sync.dma_start(
                        out=y[:, :tlen], in_=yb_d[:, toff : toff + tlen]
                    )
                else:
                    # Y = X * tgt_b  (per-slot columns, per-partition scalar)
                    for s in range(sa, sb):
                        a = int(offs[s] - toff)
                        b = int(offs[s + 1] - toff)
                        nc.gpsimd.tensor_scalar_mul(
                            y[:, a:b], x[:, a:b].bitcast(F32), tgt[:, s : s + 1]
                        )

                ntiles = (tlen + TILE_N - 1) // TILE_N
                for j in range(ntiles):
                    c0 = j * TILE_N
                    c1 = min(tlen, c0 + TILE_N)
                    n = c1 - c0
                    # slot segments covered by this tile (chunk-local cols)
                    segs = []
                    for s in range(sa, sb):
                        a = max(int(offs[s] - toff), c0)
                        b = min(int(offs[s + 1] - toff), c1)
                        if a < b:
                            segs.append((s, a, b))

                    zz = []
                    for h in (0, 1):
                        z = pspool.tile([D, TILE_N], F32, tag=f"z{h}")
                        hs = slice(h * D, h * D + D)
                        nc.tensor.matmul(
                            z[:, :n],
                            wbc[:, hs],
                            x[:, c0:c1],
                            start=True,
                            stop=False,
                        )
                        if HOST_Y_BF16:
                            nc.tensor.matmul(
                                z[:, :n],
                                wd[:, hs],
                                y[:, c0:c1],
                                start=False,
                                stop=False,
                            )
                        else:
                            nc.tensor.matmul(
                                z[:, :n],
                                wd[:, hs].bitcast(F32R),
                                y[:, c0:c1].bitcast(F32R),
                                start=False,
                                stop=False,
                            )
                        # per-slot bias via 32-slot window indicator matmul
                        wins = {}
                        for s, a, b in segs:
                            w = s // 32
                            if w in wins:
                                lo, hi = wins[w]
                                wins[w] = (min(lo, a), max(hi, b))
                            else:
                                wins[w] = (a, b)
                        witems = sorted(wins.items())
                        for wi, (w, (a, b)) in enumerate(witems):
                            nc.tensor.matmul(
                                z[:, a - c0 : b - c0],
                                cbw[
                                    :, w * HID + h * D : w * HID + h * D + D
                                ],
                                indt[:, a:b],
                                start=False,
                                stop=(wi == len(witems) - 1),
                            )
                        zz.append(z)

                    r0 = rpool.tile([D, TILE_N], RDT, tag="r0")
                    r1 = rpool.tile([D, TILE_N], RDT, tag="r1")
                    nc.scalar.activation(
                        r0[:, :n], zz[0][:, :n], mybir.ActivationFunctionType.Relu
                    )
                    nc.scalar.activation(
                        r1[:, :n], zz[1][:, :n], mybir.ActivationFunctionType.Relu
                    )

                    # P[:, t] = score(t) + b2 on every partition
                    pbc = pspool.tile([D, TILE_N], F32, tag="pbc")
                    if RELU_BF16:
                        w2r0, w2r1 = w2r[:, 0:D], w2r[:, D:HID]
                        rr0, rr1 = r0[:, :n], r1[:, :n]
                    else:
                        w2r0 = w2r[:, 0:D].bitcast(F32R)
                        w2r1 = w2r[:, D:HID].bitcast(F32R)
                        rr0 = r0[:, :n].bitcast(F32R)
                        rr1 = r1[:, :n].bitcast(F32R)
                    nc.tensor.matmul(pbc[:, :n], w2r0, rr0, start=True, stop=False)
                    nc.tensor.matmul(pbc[:, :n], w2r1, rr1, start=False, stop=True)

                    dump = dpool.tile([D, TILE_N], F32, tag="dump")
                    if REDUCE_MODE == "ttr":
                        for s, a, b in segs:
                            first = a == int(offs[s] - toff)
                            nc.vector.tensor_tensor_reduce(
                                out=dump[:, a - c0 : b - c0],
                                in0=(x[:, a:b] if XB_BF16 else x[:, a:b].bitcast(F32)),
                                in1=pbc[:, a - c0 : b - c0],
                                scale=1.0,
                                scalar=0.0 if first else acc[:, s : s + 1],
                                op0=mybir.AluOpType.mult,
                                op1=mybir.AluOpType.add,
                                accum_out=acc[:, s : s + 1],
                            )
                    elif REDUCE_MODE == "stt":
                        for s, a, b in segs:
                            first = a == int(offs[s] - toff)
                            tgt_col = (
                                acc[:, s : s + 1]
                                if first
                                else aux[:, 0:1]
                            )
                            nc.vector.scalar_tensor_tensor(
                                out=dump[:, a - c0 : b - c0],
                                in0=pbc[:, a - c0 : b - c0],
                                scalar=B2VAL[0],
                                in1=(
                                    x[:, a:b]
                                    if XB_BF16
                                    else x[:, a:b].bitcast(F32)
                                ),
                                op0=mybir.AluOpType.add,
                                op1=mybir.AluOpType.mult,
                                accum_out=tgt_col,
                            )
                            if not first:
                                nc.vector.tensor_add(
                                    acc[:, s : s + 1],
                                    acc[:, s : s + 1],
                                    aux[:, 0:1],
                                )
                    else:  # "ttred"
                        nc.vector.tensor_tensor(
                            out=dump[:, :n],
                            in0=(x[:, c0:c1] if XB_BF16 else x[:, c0:c1].bitcast(F32)),
                            in1=pbc[:, :n],
                            op=mybir.AluOpType.mult,
                        )
                        for s, a, b in segs:
                            first = a == int(offs[s] - toff)
                            tgt_col = (
                                acc[:, s : s + 1] if first else aux[:, 0:1]
                            )
                            nc.vector.tensor_reduce(
                                out=tgt_col,
                                in_=dump[:, a - c0 : b - c0],
                                axis=mybir.AxisListType.X,
                                op=mybir.AluOpType.add,
                            )
                            if not first:
                                nc.vector.tensor_add(
                                    acc[:, s : s + 1],
                                    acc[:, s : s + 1],
                                    aux[:, 0:1],
                                )

            nc.sync.dma_start(out=out_d[:], in_=acc[:])
    nc.compile()
    return nc


def _pack_core(item_seq, target, cmat, nvec, order, slot_lens, offs, T, core):
    S = len(slot_lens)
    NW = (S + 31) // 32
    x_nat = np.zeros((T, D), dtype=np.float32)
    y_nat = np.zeros((T, D), dtype=np.float32) if HOST_Y_BF16 else None
    from ml_dtypes import bfloat16

    ind = np.zeros((32, T), dtype=bfloat16)
    tgt = np.zeros((D, S), dtype=np.float32)
    cbw = np.zeros((32, NW * HID), dtype=bfloat16)
    for s in range(S):
        b = int(order[N_CORES * s + core])
        o = int(offs[s])
        nb = int(nvec[b])
        if nb > 0:
            x_nat[o : o + nb] = item_seq[b, :nb]
            if y_nat is not None:
                y_nat[o : o + nb] = item_seq[b, :nb] * target[b]
        ind[s % 32, o : o + slot_lens[s]] = 1.0
        tgt[:, s] = target[b]
        cbw[s % 32, (s // 32) * HID : (s // 32 + 1) * HID] = cmat[b]
    xt = np.ascontiguousarray(x_nat.T)
    if XB_BF16:
        from ml_dtypes import bfloat16

        xt = xt.astype(bfloat16)
    m = {"xt": xt, "ind": ind, "cbw": cbw}
    if HOST_Y_BF16:
        from ml_dtypes import bfloat16

        m["yb"] = np.ascontiguousarray(y_nat.T).astype(bfloat16)
    else:
        m["tgt"] = tgt
    return m


def build_all(target, item_seq, seq_len, W1, b1, W2, b2):
    """Build (nc, in_maps, assemble) without running — used by kernel()
    and by test harnesses that want to run/profile the program."""
    target = np.asarray(target, dtype=np.float32)
    item_seq = np.asarray(item_seq, dtype=np.float32)
    W1 = np.asarray(W1, dtype=np.float32)
    b1 = np.asarray(b1, dtype=np.float32)
    W2 = np.asarray(W2, dtype=np.float32)
    b2 = np.asarray(b2, dtype=np.float32)

    nvec, order, slot_lens, offs, T, chunks = _plan(seq_len)
    S = len(slot_lens)

    W1a, W1b = W1[0:D], W1[D : 2 * D]
    W1c, W1d = W1[2 * D : 3 * D], W1[3 * D : 4 * D]
    wbc = np.ascontiguousarray(W1b + W1c)
    wd = np.ascontiguousarray(W1d)
    cmat = (target @ (W1a - W1c) + b1).astype(np.float32)  # (B, 256)
    w2r = np.empty((D, HID), dtype=np.float32)
    w2r[:, 0:D] = np.repeat(W2[0:D, 0:1], D, axis=1)  # [k, m] = W2[k]
    w2r[:, D:HID] = np.repeat(W2[D:HID, 0:1], D, axis=1)
    B2VAL[0] = float(np.asarray(b2).reshape(-1)[0])

    if HOST_Y_BF16 or RELU_BF16:
        from ml_dtypes import bfloat16
    if HOST_Y_BF16:
        wd = wd.astype(bfloat16)
    if XB_BF16:
        wbc = wbc.astype(bfloat16)
    if RELU_BF16:
        w2r = w2r.astype(bfloat16)

    nc = _build_program(slot_lens, offs, T, chunks)

    shared = {"wbc": wbc, "wd": wd, "w2r": w2r}
    in_maps = []
    for k in range(N_CORES):
        m = _pack_core(item_seq, target, cmat, nvec, order, slot_lens, offs, T, k)
        m.update(shared)
        in_maps.append(m)

    def assemble(results):
        out = np.zeros((B_FULL, D), dtype=np.float32)
        for k in range(N_CORES):
            ot = np.asarray(results[k]["out_t"])  # (128, 256)
            for s in range(S):
                out[int(order[N_CORES * s + k])] = ot[:, s]
        return out

    return nc, in_maps, assemble


def kernel(target, item_seq, seq_len, W1, b1, W2, b2):
    nc, in_maps, assemble = build_all(target, item_seq, seq_len, W1, b1, W2, b2)
    res = run_bass_kernel_spmd(nc, in_maps, list(range(N_CORES)))
    results = res.results if hasattr(res, "results") else res
    return assemble(results)



# revision 5
# speedup vs baseline: 1.0432x; 1.0432x over previous
"""DIN-attention kernel for Trainium2, 8-core SPMD.

Reference computation (per batch b, seq pos l, x = item_seq[b, l]):
    mlp_in = [tgt, x, x-tgt, x*tgt]           (4D = 512)
    h      = relu(mlp_in @ W1 + b1)           (2D = 256)
    score  = h @ W2 + b2                      (1)
    out_b  = sum_l score[l] * x[l] * (l < seq_len[b])

Algebraic restructure (W1 = [A; B; C; Dm] in 128-row blocks):
    z   = x @ (B + C) + (x*tgt) @ Dm + c_b,   c_b = tgt_b @ (A - C) + b1
    out = sum_{l < n_b} (W2.T relu(z) + b2) * x[l]

Layout: batches sorted by seq_len descending; slot s holds global ranks
[8s, 8s+8), one per core, padded to a shared even length, so all 8
cores run one SPMD program over a packed token stream of ~25.9k valid
tokens (half the dense count).

Engine plan per core (PE ~66us is the bottleneck; everything else hides
behind it):
  - DMA: x ships once as a fused per-piece byte blob [bf16 | fp8] plus a
    0/1 fp8 slot-window indicator; a few large pieces sized so transfers
    stay ahead of compute.  Weights ride in one blob.  y for the trailing
    zone comes precomputed via DMA.
  - Pool/GpSimd: y = x * tgt_s per slot (per-partition-scalar multiply).
  - PE per 512-token tile, 6 matmuls into one combined z PSUM tile
    [D, 1024] (h0|h1):
      * wd_h.T y          (bf16, 1 col/cycle)
      * X-term as ONE fp8 DoubleRow matmul per half: planes
        (wbc_hi, wbc_lo) x (x8, x8 broadcast) -> exact-weight fp8-x
        product at 0.5 col/cycle (x8 single fp8 costs ~1.9e-2 rel err,
        inside the 2e-2 budget)
      * c-bias as ONE fp8 DoubleRow matmul per half: planes
        (cbw_hi, cbw_lo) x (ind, ind); ind is exact in fp8 and hi+lo
        splits the f32 bias rows to ~1e-3
  - ACT: ONE relu over the combined tile -> r bf16 (tail tiles pack h1
    right after h0 so only written columns are touched).
  - PE: score broadcast P[:, tile] = w2rep0.T r0 + w2rep1.T r1 into a
    per-chunk PSUM buffer (w2rep replicates W2 so every row holds the
    scalar score).
  - DVE: per tile, one full-width stt dump = (P + b2) * x -> bf16 SBUF;
    per slot, one all-SBUF bf16 tensor_scalar accumulate (4x DVE mode)
    into acc[:, s]; short tail slots are padded to group-uniform lengths
    and reduced 8-at-a-time with a single strided tensor_reduce.
  - Software pipelining: PE runs two tiles ahead of the deferred score
    matmuls (chunk boundaries included) so relu latency never stalls it;
    chunks taper small at both ends; dummy warm-up matmuls ramp the PE
    clock out of its low p-state while the first DMAs land.
"""

import sys

import numpy as np

for _p in ("/opt/trn_rl_repo",):
    if _p not in sys.path:
        sys.path.insert(0, _p)

import concourse.bacc as bacc
import concourse.bass as bass
import concourse.tile as tile
from concourse import mybir
from concourse.bass_utils import run_bass_kernel_spmd

assert bass  # re-exported for callers

B_FULL = 2048
L_FULL = 200
D = 128
N_CORES = 8
HID = 256  # 2D
TILE_N = 512  # fp32 PSUM bank columns
CHUNK_TARGET = 512  # tokens per chunk (PSUM P buffer width)
FIRST_CHUNK = 256
TAIL_ZONE = 2048  # last tokens packed into small chunks
YD_ZONE = 4096  # trailing tokens whose y ships via DMA instead of Pool
GROUP_R = 8  # tail slots per grouped reduce
TAIL_CHUNK = 256
PIECE_TARGETS = [1024, 2048, 4096]  # then 8192 each
PIECE_MAX = 8192
F32 = mybir.dt.float32
BF16 = mybir.dt.bfloat16
FP8 = mybir.dt.float8e4

Y_MODE = "pool"  # "pool" | "dma"
Y_DVE_MOD = 0  # slots with s % MOD == 0 build y on DVE (4x), rest on Pool
X_FP8 = True  # z X-term via fp8 DoubleRow (x single-fp8, wbc hi/lo planes)
B2VAL = [0.0]  # b2 constant, set by build_all before tracing
TTAIL0 = [0]  # tail-zone start token, set by build_all before tracing


def _plan(seq_len):
    """Slot/chunk/piece plan shared by all cores (SPMD)."""
    n = np.clip(np.asarray(seq_len).astype(np.int64), 0, L_FULL)
    order = np.argsort(-n, kind="stable")  # descending
    n_sorted = n[order]
    slot_lens = []
    for s in range(B_FULL // N_CORES):
        m = int(n_sorted[N_CORES * s])  # max of ranks [8s, 8s+8)
        if m <= 0:
            break
        slot_lens.append(m + (m & 1))  # round up to even
    S = len(slot_lens)
    # group + pad the tail slots (suffix with total <= TAIL_ZONE) so one
    # grouped tensor_reduce handles up to GROUP_R of them
    suf = 0
    k0 = S
    while k0 > 0 and suf + slot_lens[k0 - 1] <= TAIL_ZONE:
        suf += slot_lens[k0 - 1]
        k0 -= 1
    suf = 0
    ky0 = S
    while ky0 > 0 and suf + slot_lens[ky0 - 1] <= YD_ZONE:
        suf += slot_lens[ky0 - 1]
        ky0 -= 1
    groups = {}  # slot_start -> (count, padded_len)
    unit_starts = list(range(k0))
    s = k0
    while s < S:
        g = min(GROUP_R, S - s)
        while g > 1 and g * max(slot_lens[s : s + g]) > TILE_N:
            g -= 1
        unit_starts.append(s)
        if g > 1:
            m = max(slot_lens[s : s + g])
            for t_ in range(s, s + g):
                slot_lens[t_] = m
            groups[s] = (g, m)
        s += g
    offs = np.zeros(S + 1, dtype=np.int64)
    offs[1:] = np.cumsum(slot_lens)
    T = int(offs[-1])

    # chunk over units (a group is atomic)
    starts = sorted(set(unit_starts + [S]))
    chunks = []  # (slot_a, slot_b, tok_off, tok_len)
    ui = 0
    while ui < len(starts) - 1:
        sa = starts[ui]
        if not chunks:
            cap = FIRST_CHUNK
        elif T - int(offs[sa]) <= TAIL_ZONE:
            cap = TAIL_CHUNK
        else:
            cap = CHUNK_TARGET
        uj = ui
        while uj < len(starts) - 1 and offs[starts[uj + 1]] - offs[sa] <= cap:
            uj += 1
        if uj == ui:
            uj = ui + 1
        sb = starts[uj]
        chunks.append((sa, sb, int(offs[sa]), int(offs[sb] - offs[sa])))
        ui = uj

    pieces = []  # (tok_off, tok_len)
    chunk_piece = []  # chunk idx -> piece idx
    ci = 0
    pi = 0
    while ci < len(chunks):
        cap = PIECE_TARGETS[pi] if pi < len(PIECE_TARGETS) else PIECE_MAX
        start = chunks[ci][2]
        cj = ci
        while cj < len(chunks) and chunks[cj][2] + chunks[cj][3] - start <= cap:
            chunk_piece.append(pi)
            cj += 1
        if cj == ci:
            chunk_piece.append(pi)
            cj = ci + 1
        end = chunks[cj - 1][2] + chunks[cj - 1][3]
        pieces.append((start, end - start))
        ci = cj
        pi += 1
    return n, order, slot_lens, offs, T, chunks, pieces, chunk_piece, groups, ky0


def _build_program(slot_lens, offs, T, chunks, pieces, chunk_piece, groups, k0):
    S = len(slot_lens)
    NW = (S + 31) // 32  # 32-slot bias windows
    nc = bacc.Bacc("TRN2", target_bir_lowering=False, debug=False)

    U8 = mybir.dt.uint8
    XROW = 3 if X_FP8 else 2  # bytes per token per partition in xx blob
    xx_d = nc.dram_tensor("xx", [D, XROW * T], U8, kind="ExternalInput")
    # wblob = [wcat bf16 | wbc8 fp8 | tgt f32] as raw bytes
    WCB = 2 * (3 * HID)
    W8B = (2 * HID) if X_FP8 else 0
    wblob_d = nc.dram_tensor("wblob", [D, WCB + W8B], U8, kind="ExternalInput")
    tgt_d = nc.dram_tensor("tgt", [D, max(S, 1)], F32, kind="ExternalInput")
    ind_d = nc.dram_tensor("ind", [32, 2, T], FP8, kind="ExternalInput")
    cbw_d = nc.dram_tensor("cbw", [32, 2, NW * HID], FP8, kind="ExternalInput")

    if Y_MODE == "dma":
        yb_d = nc.dram_tensor("yb", [D, T], BF16, kind="ExternalInput")
    ytail_d = nc.dram_tensor("ytail", [D, max(T - TTAIL0[0], 1)], BF16,
                             kind="ExternalInput")
    out_d = nc.dram_tensor("out_t", [D, 256], F32, kind="ExternalOutput")

    with tile.TileContext(nc) as tc:
        with (
            tc.tile_pool(name="const", bufs=1) as cpool,
            tc.tile_pool(name="res", bufs=1) as respool,
            tc.tile_pool(name="yst", bufs=4) as ypool,
            tc.tile_pool(name="rst", bufs=4) as rpool,
            tc.tile_pool(name="dst", bufs=3) as dpool,
            tc.tile_pool(name="kst", bufs=2) as kpool,
            tc.tile_pool(name="zps", bufs=3, space="PSUM") as zpool,
            tc.tile_pool(name="pps", bufs=2, space="PSUM") as ppool,
        ):
            wblob = cpool.tile([D, WCB + W8B], U8, tag="wblob")
            tgt = cpool.tile([D, max(S, 1)], F32, tag="tgt")
            wcat = wblob[:, 0:WCB].bitcast(BF16)
            if X_FP8:
                wbc8 = (
                    wblob[:, WCB : WCB + W8B]
                    .bitcast(FP8)
                    .rearrange("p (two f) -> p two f", two=2)
                )
            cbw = cpool.tile([32, 2, NW * HID], FP8, tag="cbw")
            acc = cpool.tile([D, 256], F32, tag="acc")

            xxs, xts, inds, ybs, x8s = [], [], [], [], []
            for i, (poff, plen) in enumerate(pieces):
                xxp = respool.tile([D, XROW * plen], U8, tag=f"xx{i}", name=f"xx{i}")
                ip = respool.tile([32, 2, plen], FP8, tag=f"i{i}", name=f"i{i}")
                xxs.append(xxp)
                xts.append(xxp[:, 0 : 2 * plen].bitcast(BF16))
                inds.append(ip)
                if X_FP8:
                    x8s.append(xxp[:, 2 * plen : 3 * plen].bitcast(FP8))
                if Y_MODE == "dma":
                    yp = respool.tile([D, plen], BF16, tag=f"y{i}", name=f"y{i}")
                    ybs.append(yp)

            def dma_piece(i):
                poff, plen = pieces[i]
                nc.sync.dma_start(
                    out=xxs[i][:],
                    in_=xx_d[:, XROW * poff : XROW * (poff + plen)],
                )
                nc.sync.dma_start(
                    out=inds[i][:], in_=ind_d[:, :, poff : poff + plen]
                )
                if Y_MODE == "dma":
                    nc.sync.dma_start(
                        out=ybs[i][:], in_=yb_d[:, poff : poff + plen]
                    )

            # piece 0 first so compute can start; consts next; rest after
            poff0, plen0 = pieces[0]
            nc.sync.dma_start(
                out=xxs[0][:], in_=xx_d[:, XROW * poff0 : XROW * (poff0 + plen0)]
            )
            nc.sync.dma_start(out=wblob[:], in_=wblob_d[:])
            nc.gpsimd.dma_start(out=tgt[:], in_=tgt_d[:])
            nc.sync.dma_start(out=inds[0][:], in_=ind_d[:, :, poff0 : poff0 + plen0])
            nc.gpsimd.dma_start(out=cbw[:], in_=cbw_d[:])
            if Y_MODE == "dma":
                nc.sync.dma_start(out=ybs[0][:], in_=yb_d[:, poff0 : poff0 + plen0])
            nc.vector.memset(acc[:], 0.0)
            wu_w = cpool.tile([D, D], BF16, tag="wu_w")
            wu_x = cpool.tile([D, TILE_N], BF16, tag="wu_x")
            nc.vector.memset(wu_w[:], 0.0)
            nc.vector.memset(wu_x[:], 0.0)
            wu_z = zpool.tile([D, 2 * TILE_N], F32, tag="z", name="wu_z")
            for _ in range(3):
                nc.tensor.matmul(
                    wu_z[:, :TILE_N], wu_w[:], wu_x[:], start=True, stop=True
                )
            for i in range(1, len(pieces)):
                dma_piece(i)

            wbc = wcat[:, 0:HID]
            wd = wcat[:, HID : 2 * HID]
            w2r = wcat[:, 2 * HID : 3 * HID]

            S_ = len(slot_lens)
            ttail0 = int(offs[k0]) if k0 < S_ else T
            tail_len = T - ttail0
            y_tail = None
            if Y_MODE != "dma" and tail_len > 0:
                y_tail = cpool.tile(
                    [D, tail_len], BF16, tag="y_tail", name="y_tail"
                )
                nc.sync.dma_start(out=y_tail[:], in_=ytail_d[:, :tail_len])

            tiles = []  # (ci, c0, c1, is_last_tile_of_chunk)
            for ci, (sa, sb, toff, tlen) in enumerate(chunks):
                ntiles = (tlen + TILE_N - 1) // TILE_N
                for j in range(ntiles):
                    c0 = j * TILE_N
                    c1 = min(tlen, c0 + TILE_N)
                    tiles.append((ci, c0, c1, j == ntiles - 1))

            P_of, dumps, yloc_of, xp_of = {}, {}, {}, {}
            pend = []  # [(ci, r, c0, c1, is_last)]

            def flush_one():
                ci, r, c0, c1, is_last, zoff1 = pend.pop(0)
                sa, sb, toff, tlen = chunks[ci]
                xo = toff - pieces[chunk_piece[ci]][0]
                P = P_of[ci]
                n = c1 - c0
                nc.tensor.matmul(
                    P[:, c0:c1], w2r[:, 0:D], r[:, :n],
                    start=True, stop=False,
                )
                nc.tensor.matmul(
                    P[:, c0:c1], w2r[:, D:HID], r[:, zoff1 : zoff1 + n],
                    start=False, stop=True,
                )
                dump = dumps[ci]
                nc.vector.scalar_tensor_tensor(
                    out=dump[:, c0:c1],
                    in0=P[:, c0:c1],
                    scalar=B2VAL[0],
                    in1=xp_of[ci][:, xo + c0 : xo + c1],
                    op0=mybir.AluOpType.add,
                    op1=mybir.AluOpType.mult,
                )
                if is_last:
                    junk = kpool.tile([D, CHUNK_TARGET], BF16, tag="junk")
                    s = sa
                    while s < sb:
                        a = int(offs[s] - toff)
                        if s in groups:
                            g, gl = groups[s]
                            nc.vector.tensor_reduce(
                                out=acc[:, s : s + g],
                                in_=dump[:, a : a + g * gl].rearrange(
                                    "p (g l) -> p g l", g=g
                                ),
                                axis=mybir.AxisListType.X,
                                op=mybir.AluOpType.add,
                            )
                            s += g
                            continue
                        b = int(offs[s + 1] - toff)
                        nc.vector.tensor_scalar(
                            out=junk[:, a:b],
                            in0=dump[:, a:b],
                            scalar1=0.0,
                            scalar2=0.0,
                            op0=mybir.AluOpType.add,
                            op1=mybir.AluOpType.add,
                            accum_out=acc[:, s : s + 1],
                        )
                        s += 1
                    del P_of[ci], dumps[ci]

            tiles_left = len(tiles)
            for ci, c0, c1, is_last in tiles:
                tiles_left -= 1
                sa, sb, toff, tlen = chunks[ci]
                pi = chunk_piece[ci]
                poff, _ = pieces[pi]
                xo = toff - poff
                xp = xts[pi]
                ip = inds[pi]

                if ci not in P_of:
                    xp_of[ci] = xp
                    P_of[ci] = ppool.tile([D, CHUNK_TARGET], F32, tag="P", name=f"P{ci}")
                    dumps[ci] = dpool.tile([D, CHUNK_TARGET], BF16, tag="dump", name=f"dump{ci}")
                    if Y_MODE != "dma" and y_tail is not None and toff >= ttail0:
                        yloc_of[ci] = None  # uses y_tail
                    elif Y_MODE != "dma":
                        yloc = ypool.tile([D, CHUNK_TARGET], BF16, tag="y")
                        for s in range(sa, sb):
                            a = int(offs[s] - toff)
                            b = int(offs[s + 1] - toff)
                            eng = (
                                nc.vector
                                if (Y_DVE_MOD and s % Y_DVE_MOD == 0)
                                else nc.gpsimd
                            )
                            eng.tensor_scalar_mul(
                                yloc[:, a:b],
                                xp[:, xo + a : xo + b],
                                tgt[:, s : s + 1],
                            )
                        yloc_of[ci] = yloc

                n = c1 - c0
                wins = {}
                for s in range(sa, sb):
                    a = max(int(offs[s] - toff), c0)
                    b = min(int(offs[s + 1] - toff), c1)
                    if a < b:
                        w = s // 32
                        if w in wins:
                            lo, hi = wins[w]
                            wins[w] = (min(lo, a), max(hi, b))
                        else:
                            wins[w] = (a, b)
                witems = sorted(wins.items())

                z = zpool.tile([D, 2 * TILE_N], F32, tag="z")
                zoff1 = n if n <= 256 else TILE_N
                for h in (0, 1):
                    zc = h * zoff1
                    hs = slice(h * D, h * D + D)
                    if Y_MODE == "dma":
                        nc.tensor.matmul(
                            z[:, zc : zc + n], wd[:, hs],
                            ybs[pi][:, xo + c0 : xo + c1],
                            start=True, stop=False,
                        )
                    elif yloc_of.get(ci) is None:
                        nc.tensor.matmul(
                            z[:, zc : zc + n], wd[:, hs],
                            y_tail[:, toff - ttail0 + c0 : toff - ttail0 + c1],
                            start=True, stop=False,
                        )
                    else:
                        nc.tensor.matmul(
                            z[:, zc : zc + n], wd[:, hs],
                            yloc_of[ci][:, c0:c1],
                            start=True, stop=False,
                        )
                    if X_FP8:
                        x8b = (
                            x8s[pi][:, xo + c0 : xo + c1]
                            .unsqueeze(1)
                            .broadcast_to([D, 2, n])
                        )
                        nc.tensor.matmul(
                            z[:, zc : zc + n],
                            wbc8[:, :, h * D : h * D + D],
                            x8b,
                            start=False, stop=False,
                            perf_mode=mybir.MatmulPerfMode.DoubleRow,
                        )
                    else:
                        nc.tensor.matmul(
                            z[:, zc : zc + n], wbc[:, hs],
                            xp[:, xo + c0 : xo + c1],
                            start=False, stop=False,
                        )
                    for wi, (w, (a, b)) in enumerate(witems):
                        co = w * HID + h * D
                        nc.tensor.matmul(
                            z[:, zc + a - c0 : zc + b - c0],
                            cbw[:, :, co : co + D],
                            ip[:, :, xo + a : xo + b],
                            start=False,
                            stop=(wi == len(witems) - 1),
                            perf_mode=mybir.MatmulPerfMode.DoubleRow,
                        )

                r = rpool.tile([D, 2 * TILE_N], BF16, tag="r")
                nc.scalar.activation(
                    r[:, : zoff1 + n], z[:, : zoff1 + n],
                    mybir.ActivationFunctionType.Relu,
                )
                pend.append((ci, r, c0, c1, is_last, zoff1))
                depth = 2
                while len(pend) > depth:
                    flush_one()
            while pend:
                flush_one()

            nc.sync.dma_start(out=out_d[:], in_=acc[:])
    nc.compile()
    return nc


def _pack_core(item_seq, target, nvec, order, slot_lens, offs, T, core):
    from ml_dtypes import bfloat16

    S = len(slot_lens)
    x_nat = np.zeros((T, D), dtype=np.float32)
    y_nat = np.zeros((T, D), dtype=np.float32) if Y_MODE == "dma" else None
    tgt = np.zeros((D, max(S, 1)), dtype=np.float32)
    for s in range(S):
        b = int(order[N_CORES * s + core])
        o = int(offs[s])
        nb = int(nvec[b])
        if nb > 0:
            x_nat[o : o + nb] = item_seq[b, :nb]
            if y_nat is not None:
                y_nat[o : o + nb] = item_seq[b, :nb] * target[b]
        tgt[:, s] = target[b]
    xtb = np.ascontiguousarray(x_nat.T).astype(bfloat16)
    m = {
        "_xtb": xtb,
        "tgt": tgt,
    }
    if X_FP8:
        from ml_dtypes import float8_e4m3 as _f8
        m["_x8"] = np.ascontiguousarray(x_nat.T).astype(_f8)
    ttail0 = TTAIL0[0]
    yt = np.zeros((max(T - ttail0, 1), D), dtype=np.float32)
    for s in range(S):
        o = int(offs[s])
        if o < ttail0:
            continue
        b = int(order[N_CORES * s + core])
        nb = int(nvec[b])
        if nb > 0:
            yt[o - ttail0 : o - ttail0 + nb] = x_nat[o : o + nb] * target[b]
    m["ytail"] = np.ascontiguousarray(yt.T).astype(bfloat16)
    if y_nat is not None:
        m["yb"] = np.ascontiguousarray(y_nat.T).astype(bfloat16)
    return m


def build_all(target, item_seq, seq_len, W1, b1, W2, b2):
    """Build (nc, in_maps, assemble) without running."""
    from ml_dtypes import bfloat16, float8_e4m3

    target = np.asarray(target, dtype=np.float32)
    item_seq = np.asarray(item_seq, dtype=np.float32)
    W1 = np.asarray(W1, dtype=np.float32)
    b1 = np.asarray(b1, dtype=np.float32)
    W2 = np.asarray(W2, dtype=np.float32)
    b2 = np.asarray(b2, dtype=np.float32)

    nvec, order, slot_lens, offs, T, chunks, pieces, chunk_piece, groups, k0 = _plan(seq_len)
    S = len(slot_lens)
    TTAIL0[0] = int(offs[k0]) if k0 < S else T
    NW = (S + 31) // 32

    W1a, W1b = W1[0:D], W1[D : 2 * D]
    W1c, W1d = W1[2 * D : 3 * D], W1[3 * D : 4 * D]
    wcat3 = np.empty((D, 3 * HID), dtype=np.float32)
    wcat3[:, 0:HID] = W1b + W1c
    wcat3[:, HID : 2 * HID] = W1d
    wcat3[:, 2 * HID : 2 * HID + D] = np.repeat(W2[0:D, 0:1], D, axis=1)
    wcat3[:, 2 * HID + D : 3 * HID] = np.repeat(W2[D:HID, 0:1], D, axis=1)
    cmat = (target @ (W1a - W1c) + b1).astype(np.float32)  # (B, 256)
    B2VAL[0] = float(np.asarray(b2).reshape(-1)[0])

    ind1 = np.zeros((32, T), dtype=np.float32)
    for s in range(S):
        ind1[s % 32, int(offs[s]) : int(offs[s + 1])] = 1.0
    ind8 = np.ascontiguousarray(
        np.stack([ind1, ind1], axis=1).astype(float8_e4m3)
    )

    nc = _build_program(slot_lens, offs, T, chunks, pieces, chunk_piece, groups, k0)

    in_maps = []
    for k in range(N_CORES):
        m = _pack_core(item_seq, target, nvec, order, slot_lens, offs, T, k)
        cbw = np.zeros((32, 2, NW * HID), dtype=float8_e4m3)
        for s in range(S):
            b = int(order[N_CORES * s + k])
            hi = cmat[b].astype(float8_e4m3)
            lo = (cmat[b] - hi.astype(np.float32)).astype(float8_e4m3)
            cbw[s % 32, 0, (s // 32) * HID : (s // 32 + 1) * HID] = hi
            cbw[s % 32, 1, (s // 32) * HID : (s // 32 + 1) * HID] = lo
        wb_parts = [wcat3.astype(bfloat16).view(np.uint8)]
        if X_FP8:
            wbch = wcat3[:, 0:HID].astype(float8_e4m3)
            wbcl = (wcat3[:, 0:HID] - wbch.astype(np.float32)).astype(float8_e4m3)
            wb8 = np.concatenate([wbch, wbcl], axis=1)  # [D, 2*HID] planes
            wb_parts.append(wb8.view(np.uint8))
        # assemble xx with PER-PIECE layout: [xb_piece | x8_piece] bytes
        xtb = m.pop("_xtb")
        x8a = m.pop("_x8") if X_FP8 else None
        xx_parts = []
        for poff, plen in pieces:
            xx_parts.append(xtb[:, poff : poff + plen].view(np.uint8))
            if x8a is not None:
                xx_parts.append(x8a[:, poff : poff + plen].view(np.uint8))
        m.update({
            "xx": np.ascontiguousarray(np.concatenate(xx_parts, axis=1)),
            "ind": ind8, "cbw": cbw,
            "wblob": np.ascontiguousarray(np.concatenate(wb_parts, axis=1)),
        })
        in_maps.append(m)

    def assemble(results):
        out = np.zeros((B_FULL, D), dtype=np.float32)
        for k in range(N_CORES):
            ot = np.asarray(results[k]["out_t"])  # (128, 256)
            for s in range(S):
                out[int(order[N_CORES * s + k])] = ot[:, s]
        return out

    return nc, in_maps, assemble


def kernel(target, item_seq, seq_len, W1, b1, W2, b2):
    nc, in_maps, assemble = build_all(target, item_seq, seq_len, W1, b1, W2, b2)
    res = run_bass_kernel_spmd(nc, in_maps, list(range(N_CORES)))
    results = res.results if hasattr(res, "results") else res
    return assemble(results)
